# revision 1
# baseline (speedup 1.0000x reference)
"""Bass/Trainium2 kernel for nn_BayesianBertSelfAttention (B=2,S=1024,HID=768,NH=12,HD=64).

Sharding: 24 (batch, head) pairs over 8 cores -> core c handles batch c//4,
heads {3k, 3k+1, 3k+2} with k = c%4.

Per-core pipeline (scoresT layout st[r, l]):
  P:  q/k projections as 3 column-packed bf16 matmul groups ([q0|q1], [k0|k1],
      [q2|k2]); k2 realigned to partitions 0:64 of a 4th slot by a gpsimd
      partition-moving DMA; q converted to fp8 [32,2,S] DoubleRow layout by
      gpsimd casting DMAs; v projected to va [128 r, 8 rs, 3 h, 65] bf16 with a
      ones column (softmax row sums fall out of the context matmul).
  R(h): relative-position band R'[l, c] = q . E_rev via fp8 DoubleRow matmuls
      ([32,2] contraction packing), one psum [128, 1536]; a single ACT-or-DVE
      copy converts [128,1151] f32 psum -> fp8 SBUF; DMA to a DRAM scratch with
      row pitch 1152 (Music-Transformer skew as a strided access pattern).
  S(h): skewed DMA read gives bias[l, r] blocks fp8. Per rs-tile: 8 dummy-slot
      fp8 DoubleRow matmuls against [I|0] write the TRANSPOSED bias directly
      into the score psum (start), 2 bf16 qk matmuls accumulate on top (stop).
      ACT: pg = Exp(0.125*st) bf16 (global probs, unnormalized).
      DVE: sloc2 = (st * 0.125*A) * smT  (A = 128/ln2), f32.
      Pool: bits = uint16(sloc2 + B) -> bitcast bf16 = local probs
      (Schraudolph exp; |rel err| <= ~4%, validated end-to-end).
      PE: context matmuls in [l, d] orientation: lhsT = probs block [128,128],
      rhs = va slice [128, 65] -> ctx[l-part, d] psums accumulate over rs.
  Out: unnormalized cg|cl + row sums copied psum->SBUF f32, DMA per head.
      Host normalizes (sums column) and blends with selector weights.
"""

import sys

sys.path.insert(0, "/opt/trn_rl_repo")

import math
import numpy as np
import ml_dtypes
from contextlib import ExitStack

import concourse.bass as bass
import concourse.bacc as bacc
import concourse.tile as tile
from concourse import mybir
from concourse.bass_utils import run_bass_kernel_spmd
from concourse.masks import make_identity

B, S, HID, NH, HD = 2, 1024, 768, 12, 64
MAXP = 1024
NCORES = 8
HPC = 3
LTN = S // 128
BAND = 1151
PITCH = 1152
NE = 2 * MAXP - 1

BF16 = mybir.dt.bfloat16
F32 = mybir.dt.float32
FP8 = mybir.dt.float8e4
U16 = mybir.dt.uint16
COPY = mybir.ActivationFunctionType.Copy
EXP = mybir.ActivationFunctionType.Exp
DR = mybir.MatmulPerfMode.DoubleRow

NPBF16 = ml_dtypes.bfloat16
NPFP8 = ml_dtypes.float8_e4m3

SCH_A = 128.0 / math.log(2.0)     # bf16 schraudolph slope
SCH_B = 127.0 * 128.0 - 1.0       # bf16 schraudolph intercept (tuned end-to-end)

# q/k slot layout in qkT_sb [128, 4, S]: (group, partition offset)
QG = [(0, 0), (0, 64), (2, 0)]
KG = [(1, 0), (1, 64), (3, 0)]    # k2 moved to slot 3 po 0 by a DMA

_programs = {}


def _bcast(ap, dim_count, insert_at):
    new = list(ap.ap)
    new.insert(insert_at, [0, dim_count])
    return bass.AP(tensor=ap.tensor, offset=ap.offset, ap=new)


def build_program(n_cc=6, use_m=False):
    nc = bacc.Bacc(None)
    CH = n_cc * 128

    hidT = nc.dram_tensor("hidT", [CH, S], BF16, kind="ExternalInput")
    wg = nc.dram_tensor("wg", [3, CH, 128], BF16, kind="ExternalInput")
    wv = nc.dram_tensor("wv", [CH, HPC * HD], BF16, kind="ExternalInput")
    embT2 = nc.dram_tensor("embT2", [128, NE], BF16, kind="ExternalInput")
    smT = nc.dram_tensor("smT", [S, S], BF16, kind="ExternalInput")
    if use_m:
        mvec = nc.dram_tensor("mvec", [S, 2], F32, kind="ExternalInput")
    outu = nc.dram_tensor("outu", [128, HPC, 16, 65], F32, kind="ExternalOutput")
    skews = [nc.dram_tensor(f"skew{h}", [LTN * 128 * PITCH], FP8)
             for h in range(HPC)]

    with tile.TileContext(nc) as tc, ExitStack() as ctx:
        singles = ctx.enter_context(tc.tile_pool(name="singles", bufs=1))

        hid_sb = singles.tile([128, n_cc, S], BF16)
        wg_sb = singles.tile([128, 3, n_cc, 128], BF16)
        wv_sb = singles.tile([128, n_cc, HPC * HD], BF16)
        emb_sb = singles.tile([128, NE], BF16)
        smT_sb = singles.tile([128, 8, S], BF16)
        qkT_sb = singles.tile([128, 4, S], BF16)
        va_sb = singles.tile([128, 8, HPC, 65], BF16)
        osb = singles.tile([128, HPC, 16, 65], F32)

        wg_v = wg.rearrange("g (cc p) d -> p g cc d", p=128)
        nc.sync.dma_start(out=wg_sb[:, 0], in_=wg_v[:, 0])
        hid_v = hidT.rearrange("(cc p) l -> p cc l", p=128)
        for cc in range(3):
            nc.sync.dma_start(out=hid_sb[:, cc], in_=hid_v[:, cc])
        for cc in range(3, n_cc):
            nc.sync.dma_start(out=hid_sb[:, cc], in_=hid_v[:, cc])
        nc.sync.dma_start(out=wg_sb[:, 1], in_=wg_v[:, 1])
        nc.sync.dma_start(out=wg_sb[:, 2], in_=wg_v[:, 2])
        nc.sync.dma_start(out=emb_sb, in_=embT2[:, :])
        nc.sync.dma_start(out=wv_sb,
                          in_=wv.rearrange("(cc p) d -> p cc d", p=128))
        smT_v = smT.rearrange("(rs p) l -> p rs l", p=128)
        if use_m:
            m_sb = singles.tile([128, 8, 2], F32)
            nc.sync.dma_start(out=m_sb,
                              in_=mvec.rearrange("(rs p) w -> p rs w", p=128))

        identDR = singles.tile([128, 2, 128], FP8)
        make_identity(nc, identDR[:, 0, :])
        nc.vector.memset(identDR[:, 1, :], 0.0)
        nc.vector.memset(va_sb[:, :, :, 64], 1.0)

        # Engine alternation for the 24 band->fp8 copies (0 = ACT, 1 = DVE),
        # chosen to balance total busy time across the two engines.
        rt_engine = [i % 2 for i in range(24)]

        # -------- phase P: projections + v + head-0 band; then S(h) with
        # -------- R(h+1) bands interleaved into the rs slots --------
        with tc.tile_pool(name="bskp", bufs=3) as bskp, \
             tc.tile_pool(name="rtp", bufs=2) as rtp:

            rt_i = [0]
            rts_tiles = {}
            bsk_tiles = {}

            def skew_w_view(h, lt):
                return skews[h][lt * 128 * PITCH:(lt + 1) * 128 * PITCH] \
                    .rearrange("(p c) -> p c", c=PITCH)[:, 0:BAND]

            def emit_band_lt(pool, h, lt):
                e0 = 896 - lt * 128
                # two psum tiles so the ACT and DVE fp8 conversions have one
                # reader each (readers of a tile serialize) and run parallel
                pra = pool.tile([128, 640], F32, tag="pra")
                prb = pool.tile([128, 511], F32, tag="prb")
                qg_, qpo_ = QG[h]
                qbl = qkT_sb[qpo_:qpo_ + 64, qg_, lt * 128:(lt + 1) * 128]
                for dst, c0, c1 in ((pra, 0, 512), (pra, 512, 640),
                                    (prb, 640, 1024), (prb, 1024, BAND)):
                    nc.tensor.matmul(
                        dst[:, c0 - (0 if dst is pra else 640):
                            c1 - (0 if dst is pra else 640)],
                        lhsT=qbl,
                        rhs=emb_sb[qpo_:qpo_ + 64, e0 + c0:e0 + c1],
                        start=True, stop=True,
                    )
                rts = rts_tiles[h]
                nc.scalar.activation(rts[:, lt, 0:640], pra, COPY)
                nc.vector.tensor_copy(rts[:, lt, 640:BAND], prb)
                # per-lt skew roundtrip, chasing (Pool writes, SP reads)
                nc.sync.dma_start(out=skew_w_view(h, lt), in_=rts[:, lt, :])
                rview = bass.AP(tensor=skews[h][:].tensor,
                                offset=lt * 128 * PITCH + 127,
                                ap=[[BAND, 128], [1, S]])
                nc.sync.dma_start(out=bsk_tiles[h][:, lt, :], in_=rview)

            def prep_head(h):
                rts_new = rtp.tile([128, LTN, BAND], FP8, tag="rts")
                rts_tiles[h] = rts_new
                bsk_new = bskp.tile([128, LTN, S], FP8, tag=f"bsk{h}")
                bsk_tiles[h] = bsk_new

            # ---- phase P (+R0): pt ring shared by q/k groups and v pairs ----
            with tc.tile_pool(name="ps_pt", bufs=1, space="PSUM") as ps_pt, \
                 tc.tile_pool(name="ps_pr0", bufs=2, space="PSUM") as ps_pr0:

                def emit_G_mms(g):
                    pt = ps_pt.tile([128, S], F32, tag="pt")
                    for n in range(2):
                        for cc in range(n_cc):
                            nc.tensor.matmul(
                                pt[:, n * 512:(n + 1) * 512],
                                lhsT=wg_sb[:, g, cc, :],
                                rhs=hid_sb[:, cc, n * 512:(n + 1) * 512],
                                start=(cc == 0), stop=(cc == n_cc - 1),
                            )
                    return pt

                def emit_G_copy(g, pt):
                    if g == 1:
                        nc.vector.tensor_copy(qkT_sb[:, g, :], pt)
                    else:
                        nc.scalar.activation(qkT_sb[:, g, :], pt, COPY)
                    if g == 2:
                        nc.gpsimd.dma_start(out=qkT_sb[0:64, 3, :],
                                            in_=qkT_sb[64:128, 2, :])

                def emit_vpair(rp):
                    # v pair in a pt-ring tile: regions [0:192] and [512:704]
                    pt = ps_pt.tile([128, S], F32, tag="pt")
                    for par in range(2):
                        rs = 2 * rp + par
                        for cc in range(n_cc):
                            nc.tensor.matmul(
                                pt[:, par * 512:par * 512 + HPC * HD],
                                lhsT=hid_sb[:, cc, rs * 128:(rs + 1) * 128],
                                rhs=wv_sb[:, cc, :],
                                start=(cc == 0), stop=(cc == n_cc - 1),
                            )
                    for par in range(2):
                        pslice = pt[:, par * 512:par * 512 + HPC * HD] \
                            .rearrange("p (h d) -> p h d", d=64)
                        if rp % 2 == 0:
                            nc.scalar.activation(
                                va_sb[:, 2 * rp + par, :, 0:64], pslice, COPY)
                        else:
                            nc.vector.tensor_copy(
                                va_sb[:, 2 * rp + par, :, 0:64], pslice)

                prep_head(0)
                emit_G_copy(0, emit_G_mms(0))
                emit_G_copy(1, emit_G_mms(1))
                pt_g2 = emit_G_mms(2)
                for lt in range(LTN):
                    emit_band_lt(ps_pr0, 0, lt)
                emit_G_copy(2, pt_g2)
                # smT chunks follow the bsk0 reads on the SP queue so the
                # first scores don't wait behind a bulk smT transfer
                for rsq in range(8):
                    nc.sync.dma_start(out=smT_sb[:, rsq], in_=smT_v[:, rsq])
                for rp in range(4):
                    emit_vpair(rp)

            # ------------- phase S with interleaved next-head bands -------------
            with tc.tile_pool(name="ps_st", bufs=1, space="PSUM") as ps_st, \
                 tc.tile_pool(name="ps_cx", bufs=1, space="PSUM") as ps_cx, \
                 tc.tile_pool(name="ps_pr1", bufs=1, space="PSUM") as ps_pr1, \
                 tc.tile_pool(name="pgp", bufs=6) as pgp, \
                 tc.tile_pool(name="plp", bufs=6) as plp, \
                 tc.tile_pool(name="slp", bufs=6) as slp:

                def emit_S(h):
                    bsk8 = bsk_tiles[h]
                    qg, qpo = QG[h]
                    kg, kpo = KG[h]
                    qf = qkT_sb[qpo:qpo + 64, qg, :]
                    kf = qkT_sb[kpo:kpo + 64, kg, :]
                    cx0 = ps_cx.tile([128, 7, 65], F32, tag="cx0")
                    cx1 = ps_cx.tile([128, 7, 65], F32, tag="cx1")
                    cx2 = ps_cx.tile([128, 2, 65], F32, tag="cx2")
                    cx = [cx0, cx1, cx2]
                    touched = [False, False, False]

                    def emit_ctx(rs, pg, pl):
                        for u in range(16):
                            lt, br = u // 2, u % 2
                            ti, reg = u // 7, u % 7
                            lhs = (pg if br == 0 else pl)[:, lt * 128:(lt + 1) * 128]
                            nc.tensor.matmul(
                                cx[ti][:, reg, :],
                                lhsT=lhs,
                                rhs=va_sb[:, rs, h, :],
                                start=(not touched[ti]),
                                stop=(rs == 7 and (u == 6 or u == 13 or u == 15)),
                            )
                            touched[ti] = True

                    if h + 1 < HPC:
                        prep_head(h + 1)
                    pending = {}
                    for rs in range(8):
                        pg = pgp.tile([128, S], BF16, tag="pg")
                        sloc2 = slp.tile([128, S], BF16, tag="sl")
                        for half in range(2):
                            st = ps_st.tile([128, 512], F32, tag=f"st{half}")
                            for li in range(4):
                                lt = half * 4 + li
                                nc.tensor.matmul(
                                    st[:, li * 128:(li + 1) * 128],
                                    lhsT=bsk8[:, lt, rs * 128:(rs + 1) * 128],
                                    rhs=identDR[:, 0, :],
                                    start=(li == 0), stop=False,
                                )
                            nc.tensor.matmul(
                                st,
                                lhsT=kf[:, rs * 128:(rs + 1) * 128],
                                rhs=qf[:, half * 512:(half + 1) * 512],
                                start=False, stop=True,
                            )
                            if use_m:
                                nc.vector.tensor_scalar_add(st, st, m_sb[:, rs, 0:1])
                            nc.scalar.activation(pg[:, half * 512:(half + 1) * 512],
                                                 st, EXP, scale=0.125)
                            # forward Schraudolph argument from the half psum
                            nc.vector.scalar_tensor_tensor(
                                sloc2[:, half * 512:(half + 1) * 512],
                                st, 0.125 * SCH_A,
                                smT_sb[:, rs, half * 512:(half + 1) * 512],
                                op0=mybir.AluOpType.mult,
                                op1=mybir.AluOpType.mult)

                        plb = plp.tile([128, S], U16, tag="pl")
                        if use_m:
                            nc.gpsimd.tensor_scalar_add(plb, sloc2, m_sb[:, rs, 1:2])
                        else:
                            nc.gpsimd.tensor_scalar_add(plb, sloc2, SCH_B)

                        if h + 1 < HPC:
                            emit_band_lt(ps_pr1, h + 1, rs)
                        if rs >= 2:
                            emit_ctx(rs - 2, *pending[rs - 2])
                        pending[rs] = (pg, plb[:, :].bitcast(BF16))

                    emit_ctx(6, *pending[6])
                    emit_ctx(7, *pending[7])

                    nc.scalar.activation(osb[:, h, 0:7, :], cx0, COPY)
                    nc.vector.tensor_copy(osb[:, h, 7:14, :], cx1)
                    nc.scalar.activation(osb[:, h, 14:16, :], cx2, COPY)
                    nc.sync.dma_start(out=outu[:, h], in_=osb[:, h])

                for h in range(HPC):
                    emit_S(h)

    nc.compile()
    return nc


def _get_program(n_cc, use_m):
    key = (n_cc, use_m)
    if key not in _programs:
        _programs[key] = build_program(n_cc, use_m)
    return _programs[key]


def kernel(hidden_states, attention_mask, scaled_attention_mask, selector_outputs,
           Wq, bq, Wk, bk, Wv, bv, dist_emb):
    hidden_states = np.asarray(hidden_states, np.float32)
    attention_mask = np.asarray(attention_mask, np.float32)
    scaled_attention_mask = np.asarray(scaled_attention_mask, np.float32)
    selector_outputs = np.asarray(selector_outputs, np.float32)
    Wq, Wk, Wv = (np.asarray(x, np.float32) for x in (Wq, Wk, Wv))
    bq, bk, bv = (np.asarray(x, np.float32) for x in (bq, bk, bv))
    dist_emb = np.asarray(dist_emb, np.float32)

    use_bias = bool(np.any(bq) or np.any(bk) or np.any(bv))
    use_m = bool(np.any(attention_mask))
    n_cc = 7 if use_bias else 6
    CH = n_cc * 128
    nc = _get_program(n_cc, use_m)

    smT = np.ascontiguousarray(scaled_attention_mask[0, 0].T).astype(NPBF16)
    e_rev_t = dist_emb[::-1].T.astype(np.float32)  # [64, 2047]
    embT2_np = np.ascontiguousarray(
        np.concatenate([e_rev_t, e_rev_t], axis=0)).astype(NPBF16)

    in_maps = []
    for core in range(NCORES):
        b = core // 4
        k4 = core % 4
        heads = [3 * k4, 3 * k4 + 1, 3 * k4 + 2]

        hidT = hidden_states[b].T
        if use_bias:
            hidT = np.concatenate(
                [hidT, np.ones((1, S), np.float32),
                 np.zeros((CH - HID - 1, S), np.float32)], axis=0)
        hidT_bf = np.ascontiguousarray(hidT).astype(NPBF16)

        def wcols(W, bvec, h):
            c = W[:, h * HD:(h + 1) * HD]
            if use_bias:
                c = np.concatenate(
                    [c, bvec[None, h * HD:(h + 1) * HD],
                     np.zeros((CH - HID - 1, HD), np.float32)], axis=0)
            return c

        q0, q1, q2 = (wcols(Wq, bq, h) for h in heads)
        k0, k1, k2 = (wcols(Wk, bk, h) for h in heads)
        wg_np = np.stack([
            np.concatenate([q0, q1], axis=1),
            np.concatenate([k0, k1], axis=1),
            np.concatenate([q2, k2], axis=1),
        ]).astype(NPBF16)
        wv_np = np.concatenate(
            [wcols(Wv, bv, h) for h in heads], axis=1).astype(NPBF16)

        m = {
            "hidT": hidT_bf,
            "wg": np.ascontiguousarray(wg_np),
            "wv": np.ascontiguousarray(wv_np),
            "embT2": embT2_np,
            "smT": smT,
        }
        if use_m:
            mv = attention_mask[b, 0, 0].astype(np.float32)
            m["mvec"] = np.ascontiguousarray(
                np.stack([8.0 * mv, SCH_B + SCH_A * mv], axis=1))
        in_maps.append(m)

    res = run_bass_kernel_spmd(nc, in_maps, list(range(NCORES)))

    out = np.empty((B, S, HID), np.float32)
    for core in range(NCORES):
        b = core // 4
        k4 = core % 4
        r = res.results[core]["outu"]  # [128, 3, 16, 65]
        # unit u -> (lt = u//2, br = u%2); partition p -> l = lt*128 + p
        r = r.reshape(128, HPC, 8, 2, 65)
        ctx = r[..., 0:64]                        # [128, h, lt, br, 64]
        sums = r[..., 64]                         # [128, h, lt, br]
        ctx = ctx.transpose(2, 0, 1, 3, 4).reshape(S, HPC, 2, 64)
        sums = sums.transpose(2, 0, 1, 3).reshape(S, HPC, 2)
        sel = selector_outputs[b, 0, :, 0][:, None]  # [S, 1]
        cg = ctx[:, :, 0] / sums[:, :, 0:1]
        cl = ctx[:, :, 1] / sums[:, :, 1:2]
        blended = sel[:, :, None] * cl + (1.0 - sel)[:, :, None] * cg
        out[b, :, 192 * k4:192 * (k4 + 1)] = blended.reshape(S, 192)
    return out



# revision 20
# speedup vs baseline: 1.0879x; 1.0879x over previous
"""Bass/Trainium2 kernel for nn_BayesianBertSelfAttention (B=2,S=1024,HID=768,NH=12,HD=64).

Sharding: 24 (batch, head) pairs over 8 cores -> core c handles batch c//4,
heads {3k, 3k+1, 3k+2} with k = c%4.

Per-core pipeline (scoresT layout st[r, l]):
  P:  q/k projections as 3 column-packed bf16 matmul groups ([q0|q1], [k0|k1],
      [q2|k2]); k2 realigned to partitions 0:64 of a 4th slot by a gpsimd
      partition-moving DMA; q cast to fp8 and realigned to [32,2,S] DoubleRow
      layout (2 DMAs per head); v projected to va [128 r, 8 rs, 3 h, 65] bf16
      with a ones column (softmax row sums fall out of the context matmul).
  R(h): relative-position band R'[l, c] = q . E_rev via fp8 DoubleRow matmuls
      (qDR [32,2,128] x embDR [32,2,W]), one psum pair [128,640]+[128,511];
      psum -> fp8 SBUF copies split across ACT/DVE/Pool; DMA to a DRAM scratch
      with row pitch 1152 (Music-Transformer skew as a strided access pattern).
  S(h): skewed DMA read gives bias[l, r] blocks fp8. Per rs-tile: 8 fp8
      DoubleRow matmuls against [I|0] write the TRANSPOSED bias directly into
      the score psum (start), 2 bf16 qk matmuls accumulate on top (stop).
      ACT: pg = Exp(0.125*st) bf16 (global probs, unnormalized).
      DVE: d = bits(pg) - B (i16, 4x mode); e = d * smT (i16, 2x mode).
      Pool: plb = u16(e + B) -> bitcast bf16 = local probs
      (pl = pg^m via Schraudolph bits domain; validated end-to-end ~1.1% rel).
      PE: context matmuls in [l, d] orientation: lhsT = probs block [128,128],
      rhs = va slice [128, 65] -> ctx[l-part, d] psums accumulate over rs.
  Out: unnormalized cg|cl + row sums copied psum->SBUF f16, DMA per head.
      Host normalizes (sums column) and blends with selector weights.
DMAs are deliberately spread across the SP/ACT/Pool queues (transfers from
different queues overlap in the DGE/DMA-engine model).
"""

import sys

sys.path.insert(0, "/opt/trn_rl_repo")

import math
import numpy as np
import ml_dtypes
from contextlib import ExitStack

import concourse.bass as bass
import concourse.bacc as bacc
import concourse.tile as tile
from concourse import mybir
from concourse.bass_utils import run_bass_kernel_spmd
from concourse.masks import make_identity

B, S, HID, NH, HD = 2, 1024, 768, 12, 64
MAXP = 1024
NCORES = 8
HPC = 3
LTN = S // 128
BAND = 1151
PITCH = 1152
NE = 2 * MAXP - 1
NEP = 2 * MAXP      # emb plane stride padded even: odd stride + DR ifmap crashes HW

BF16 = mybir.dt.bfloat16
F16 = mybir.dt.float16
F32 = mybir.dt.float32
FP8 = mybir.dt.float8e4
U16 = mybir.dt.uint16
I16 = mybir.dt.int16
COPY = mybir.ActivationFunctionType.Copy
EXP = mybir.ActivationFunctionType.Exp
DR = mybir.MatmulPerfMode.DoubleRow

NPBF16 = ml_dtypes.bfloat16
NPFP8 = ml_dtypes.float8_e4m3

SCH_A = 128.0 / math.log(2.0)     # bf16 schraudolph slope
SCH_B = 127.0 * 128.0             # bf16 schraudolph intercept

# q/k slot layout in qkT_sb [128, 4, S]: (group, partition offset)
QG = [(0, 0), (0, 64), (2, 0)]
KG = [(1, 0), (1, 64), (3, 0)]    # k2 moved to slot 3 po 0 by a DMA

_programs = {}


def _bcast(ap, dim_count, insert_at):
    new = list(ap.ap)
    new.insert(insert_at, [0, dim_count])
    return bass.AP(tensor=ap.tensor, offset=ap.offset, ap=new)


def build_program(n_cc=6, use_m=False):
    nc = bacc.Bacc(None)
    CH = n_cc * 128

    hidT = nc.dram_tensor("hidT", [CH, S], BF16, kind="ExternalInput")
    wg = nc.dram_tensor("wg", [3, CH, 128], BF16, kind="ExternalInput")
    wv = nc.dram_tensor("wv", [CH, HPC * HD], BF16, kind="ExternalInput")
    # fp8 E bands: plane 0 = E_rev.T rows (dup at partitions 64:128 so q1's
    # partition offset matches), plane 1 = zeros (DR pairs with bcast q)
    embDR = nc.dram_tensor("embDR", [128, 2, NEP], FP8, kind="ExternalInput")
    smT = nc.dram_tensor("smT", [S, S], BF16, kind="ExternalInput")
    if use_m:
        mvec = nc.dram_tensor("mvec", [S, 2], F32, kind="ExternalInput")
    outu = nc.dram_tensor("outu", [128, HPC, 16, 65], F16, kind="ExternalOutput")
    skews = [nc.dram_tensor(f"skew{h}", [LTN * 128 * PITCH], FP8)
             for h in range(HPC)]

    with tile.TileContext(nc) as tc, ExitStack() as ctx:
        singles = ctx.enter_context(tc.tile_pool(name="singles", bufs=1))

        hid_sb = singles.tile([128, n_cc, S], BF16)
        wg_sb = singles.tile([128, 3, n_cc, 128], BF16)
        wv_sb = singles.tile([128, n_cc, HPC * HD], BF16)
        emb_sb = singles.tile([128, 2, NEP], FP8)
        smT_sb = singles.tile([128, 8, S], BF16)
        qkT_sb = singles.tile([128, 4, S], BF16)
        qf8_sb = singles.tile([128, 2, S], FP8)   # [q0|q1] and [q2|--] fp8
        va_sb = singles.tile([128, 8, HPC, 65], BF16)
        osb = singles.tile([128, HPC, 16, 65], F16)

        wg_v = wg.rearrange("g (cc p) d -> p g cc d", p=128)
        hid_v = hidT.rearrange("(cc p) l -> p cc l", p=128)
        # ACT queue: weight groups + emb + wv (transfers overlap SP's hid)
        nc.scalar.dma_start(out=wg_sb[:, 0], in_=wg_v[:, 0])
        # SP queue: hid chunks (projection matmuls chase these)
        for cc in range(n_cc):
            nc.sync.dma_start(out=hid_sb[:, cc], in_=hid_v[:, cc])
        nc.scalar.dma_start(out=wg_sb[:, 1], in_=wg_v[:, 1])
        nc.scalar.dma_start(out=emb_sb, in_=embDR[:, :, :])
        nc.scalar.dma_start(out=wv_sb,
                            in_=wv.rearrange("(cc p) d -> p cc d", p=128))
        nc.scalar.dma_start(out=wg_sb[:, 2], in_=wg_v[:, 2])
        smT_v = smT.rearrange("(rs p) l -> p rs l", p=128)
        # smT chunks via Pool SWDGE (its queue is otherwise idle in P);
        # first rs chunks arrive well before S(0)
        for rsq in range(8):
            nc.gpsimd.dma_start(out=smT_sb[:, rsq], in_=smT_v[:, rsq])
        if use_m:
            m_sb = singles.tile([128, 8, 2], F32)
            nc.gpsimd.dma_start(out=m_sb,
                                in_=mvec.rearrange("(rs p) w -> p rs w", p=128))

        identDR = singles.tile([128, 2, 128], FP8)
        make_identity(nc, identDR[:, 0, :])
        nc.vector.memset(identDR[:, 1, :], 0.0)
        nc.vector.memset(va_sb[:, :, :, 64], 1.0)

        with tc.tile_pool(name="bskp", bufs=3) as bskp, \
             tc.tile_pool(name="rtp", bufs=2) as rtp:

            rts_tiles = {}
            bsk_tiles = {}

            # fp8 q slot for the DR band matmuls: head -> (slot, po)
            Q8 = [(0, 0), (0, 64), (1, 0)]

            def emit_band_lt(pool, h, lt, ci):
                e0 = 896 - lt * 128
                pr = pool.tile([128, BAND], F32, tag="band")
                qslot, qpo8 = Q8[h]
                qblDR = _bcast(
                    qf8_sb[qpo8:qpo8 + 64, qslot, lt * 128:(lt + 1) * 128], 2, 1)
                for c0, c1 in ((0, 512), (512, 640), (640, 1024), (1024, BAND)):
                    nc.tensor.matmul(
                        pr[:, c0:c1],
                        lhsT=qblDR,
                        rhs=emb_sb[qpo8:qpo8 + 64, :, e0 + c0:e0 + c1],
                        start=True, stop=True, perf_mode=DR,
                    )
                rts = rts_tiles[h]
                # single psum->fp8 copy per lt; rotate ACT/DVE (ACT is
                # saturated by the exps so it takes the fewest)
                if ci % 8 in (1, 4, 6):
                    nc.scalar.activation(rts[:, lt, :], pr, COPY)
                else:
                    nc.vector.tensor_copy(rts[:, lt, :], pr)
                # 2-lt-granular skew roundtrip on SP, chasing
                if lt % 2 == 1:
                    base = (lt - 1) * 128 * PITCH
                    wview = skews[h][base:base + 2 * 128 * PITCH] \
                        .rearrange("(q p c) -> p q c", c=PITCH, q=2)[:, :, 0:BAND]
                    nc.sync.dma_start(out=wview, in_=rts[:, lt - 1:lt + 1, :])
                    rview = bass.AP(tensor=skews[h][:].tensor,
                                    offset=base + 127,
                                    ap=[[BAND, 128], [128 * PITCH, 2], [1, S]])
                    nc.sync.dma_start(out=bsk_tiles[h][:, lt - 1:lt + 1, :],
                                      in_=rview)

            def prep_head(h):
                rts_new = rtp.tile([128, LTN, BAND], FP8, tag="rts")
                rts_tiles[h] = rts_new
                bsk_new = bskp.tile([128, LTN, S], FP8, tag=f"bsk{h}")
                bsk_tiles[h] = bsk_new

            # -------- phase P: projections + v + head-0 band --------
            with tc.tile_pool(name="ps_pt", bufs=1, space="PSUM") as ps_pt, \
                 tc.tile_pool(name="ps_pr0", bufs=2, space="PSUM") as ps_pr0:

                def emit_G_mms(g):
                    pt = ps_pt.tile([128, S], F32, tag="pt")
                    # cc-outer so matmuls chase the hid chunk DMAs
                    for cc in range(n_cc):
                        for n in range(2):
                            nc.tensor.matmul(
                                pt[:, n * 512:(n + 1) * 512],
                                lhsT=wg_sb[:, g, cc, :],
                                rhs=hid_sb[:, cc, n * 512:(n + 1) * 512],
                                start=(cc == 0), stop=(cc == n_cc - 1),
                            )
                    return pt

                def emit_G_copy(g, pt):
                    if g == 1:
                        nc.vector.tensor_copy(qkT_sb[:, g, :], pt)
                    else:
                        nc.scalar.activation(qkT_sb[:, g, :], pt, COPY)
                    if g == 0:
                        # q0|q1 fp8 cast for the DR band matmuls
                        nc.vector.tensor_copy(qf8_sb[:, 0, :], pt)
                    if g == 2:
                        nc.vector.tensor_copy(qf8_sb[0:64, 1, :], pt[0:64, :])
                        nc.gpsimd.dma_start(out=qkT_sb[0:64, 3, :],
                                            in_=qkT_sb[64:128, 2, :])

                def emit_vpair(rp):
                    # v pair in a pt-ring tile: regions [0:192] and [512:704]
                    pt = ps_pt.tile([128, S], F32, tag="pt")
                    for par in range(2):
                        rs = 2 * rp + par
                        for cc in range(n_cc):
                            nc.tensor.matmul(
                                pt[:, par * 512:par * 512 + HPC * HD],
                                lhsT=hid_sb[:, cc, rs * 128:(rs + 1) * 128],
                                rhs=wv_sb[:, cc, :],
                                start=(cc == 0), stop=(cc == n_cc - 1),
                            )
                    for par in range(2):
                        pslice = pt[:, par * 512:par * 512 + HPC * HD] \
                            .rearrange("p (h d) -> p h d", d=64)
                        if rp % 2 == 0:
                            nc.scalar.activation(
                                va_sb[:, 2 * rp + par, :, 0:64], pslice, COPY)
                        else:
                            nc.vector.tensor_copy(
                                va_sb[:, 2 * rp + par, :, 0:64], pslice)

                prep_head(0)
                emit_G_copy(0, emit_G_mms(0))
                emit_G_copy(1, emit_G_mms(1))
                for lt in range(LTN):
                    emit_band_lt(ps_pr0, 0, lt, lt)
                for rp in range(2):
                    emit_vpair(rp)
                emit_G_copy(2, emit_G_mms(2))
                for rp in range(2, 4):
                    emit_vpair(rp)

            # ------------- phase S with interleaved next-head bands -------------
            with tc.tile_pool(name="ps_st", bufs=1, space="PSUM") as ps_st, \
                 tc.tile_pool(name="ps_cx", bufs=1, space="PSUM") as ps_cx, \
                 tc.tile_pool(name="ps_pr1", bufs=1, space="PSUM") as ps_pr1, \
                 tc.tile_pool(name="pgp", bufs=6) as pgp, \
                 tc.tile_pool(name="dp", bufs=4) as dp, \
                 tc.tile_pool(name="plp", bufs=6) as plp:

                def emit_S(h):
                    bsk8 = bsk_tiles[h]
                    qg, qpo = QG[h]
                    kg, kpo = KG[h]
                    qf = qkT_sb[qpo:qpo + 64, qg, :]
                    kf = qkT_sb[kpo:kpo + 64, kg, :]
                    cx0 = ps_cx.tile([128, 7, 65], F32, tag="cx0")
                    cx1 = ps_cx.tile([128, 7, 65], F32, tag="cx1")
                    cx2 = ps_cx.tile([128, 2, 65], F32, tag="cx2")
                    cx = [cx0, cx1, cx2]
                    touched = [False, False, False]

                    def emit_ctx(rs, pg, pl):
                        for u in range(16):
                            lt, br = u // 2, u % 2
                            ti, reg = u // 7, u % 7
                            lhs = (pg if br == 0 else pl)[:, lt * 128:(lt + 1) * 128]
                            nc.tensor.matmul(
                                cx[ti][:, reg, :],
                                lhsT=lhs,
                                rhs=va_sb[:, rs, h, :],
                                start=(not touched[ti]),
                                stop=(rs == 7 and (u == 6 or u == 13 or u == 15)),
                            )
                            touched[ti] = True

                    if h + 1 < HPC:
                        prep_head(h + 1)
                    pending = {}
                    for rs in range(8):
                        pg = pgp.tile([128, S], BF16, tag="pg")
                        for half in range(2):
                            st = ps_st.tile([128, 512], F32, tag=f"st{half}")
                            for li in range(4):
                                lt = half * 4 + li
                                nc.tensor.matmul(
                                    st[:, li * 128:(li + 1) * 128],
                                    lhsT=_bcast(
                                        bsk8[:, lt, rs * 128:(rs + 1) * 128], 2, 1),
                                    rhs=identDR[:, :, :],
                                    start=(li == 0), stop=False, perf_mode=DR,
                                )
                            nc.tensor.matmul(
                                st,
                                lhsT=kf[:, rs * 128:(rs + 1) * 128],
                                rhs=qf[:, half * 512:(half + 1) * 512],
                                start=False, stop=True,
                            )
                            if use_m:
                                nc.vector.tensor_scalar_add(st, st, m_sb[:, rs, 0:1])
                            nc.scalar.activation(pg[:, half * 512:(half + 1) * 512],
                                                 st, EXP, scale=0.125)

                        # local branch: pl = pg^m in schraudolph bits domain;
                        # the two scalar ops alternate DVE/Pool for balance
                        d16 = dp.tile([128, S], I16, tag="d")
                        if rs % 2 == 0:
                            nc.vector.tensor_scalar_add(d16, pg[:, :].bitcast(U16),
                                                        -SCH_B)
                        else:
                            nc.gpsimd.tensor_scalar_add(d16, pg[:, :].bitcast(U16),
                                                        -SCH_B)
                        e16 = dp.tile([128, S], I16, tag="e")
                        nc.vector.tensor_tensor(e16, d16, smT_sb[:, rs, :],
                                                op=mybir.AluOpType.mult)
                        plb = plp.tile([128, S], U16, tag="pl")
                        if use_m:
                            nc.gpsimd.tensor_scalar_add(plb, e16, m_sb[:, rs, 1:2])
                        else:
                            nc.gpsimd.tensor_scalar_add(plb, e16, SCH_B)

                        # next head's band, front-loaded two lts per rs so the
                        # skew roundtrip completes well before S(h+1)
                        if h + 1 < HPC and rs < 4:
                            emit_band_lt(ps_pr1, h + 1, 2 * rs, 2 * rs + h)
                            emit_band_lt(ps_pr1, h + 1, 2 * rs + 1, 2 * rs + 1 + h)
                        if rs >= 2:
                            emit_ctx(rs - 2, *pending[rs - 2])
                        pending[rs] = (pg, plb[:, :].bitcast(BF16))

                    emit_ctx(6, *pending[6])
                    emit_ctx(7, *pending[7])

                    nc.scalar.activation(osb[:, h, 0:7, :], cx0, COPY)
                    nc.vector.tensor_copy(osb[:, h, 7:14, :], cx1)
                    nc.vector.tensor_copy(osb[:, h, 14:16, :], cx2)
                    nc.sync.dma_start(out=outu[:, h], in_=osb[:, h])

                for h in range(HPC):
                    emit_S(h)

    nc.compile()
    return nc


def _get_program(n_cc, use_m):
    key = (n_cc, use_m)
    if key not in _programs:
        _programs[key] = build_program(n_cc, use_m)
    return _programs[key]


def kernel(hidden_states, attention_mask, scaled_attention_mask, selector_outputs,
           Wq, bq, Wk, bk, Wv, bv, dist_emb):
    hidden_states = np.asarray(hidden_states, np.float32)
    attention_mask = np.asarray(attention_mask, np.float32)
    scaled_attention_mask = np.asarray(scaled_attention_mask, np.float32)
    selector_outputs = np.asarray(selector_outputs, np.float32)
    Wq, Wk, Wv = (np.asarray(x, np.float32) for x in (Wq, Wk, Wv))
    bq, bk, bv = (np.asarray(x, np.float32) for x in (bq, bk, bv))
    dist_emb = np.asarray(dist_emb, np.float32)

    use_bias = bool(np.any(bq) or np.any(bk) or np.any(bv))
    use_m = bool(np.any(attention_mask))
    n_cc = 7 if use_bias else 6
    CH = n_cc * 128
    nc = _get_program(n_cc, use_m)

    smT = np.ascontiguousarray(scaled_attention_mask[0, 0].T).astype(NPBF16)
    e_rev_t = dist_emb[::-1].T.astype(np.float32)  # [64, 2047]
    embDR_np = np.zeros((128, 2, NEP), np.float32)
    embDR_np[0:64, 0, 0:NE] = e_rev_t
    embDR_np[64:128, 0, 0:NE] = e_rev_t
    embDR_np = np.ascontiguousarray(embDR_np).astype(NPFP8)

    in_maps = []
    for core in range(NCORES):
        b = core // 4
        k4 = core % 4
        heads = [3 * k4, 3 * k4 + 1, 3 * k4 + 2]

        hidT = hidden_states[b].T
        if use_bias:
            hidT = np.concatenate(
                [hidT, np.ones((1, S), np.float32),
                 np.zeros((CH - HID - 1, S), np.float32)], axis=0)
        hidT_bf = np.ascontiguousarray(hidT).astype(NPBF16)

        def wcols(W, bvec, h):
            c = W[:, h * HD:(h + 1) * HD]
            if use_bias:
                c = np.concatenate(
                    [c, bvec[None, h * HD:(h + 1) * HD],
                     np.zeros((CH - HID - 1, HD), np.float32)], axis=0)
            return c

        q0, q1, q2 = (wcols(Wq, bq, h) for h in heads)
        k0, k1, k2 = (wcols(Wk, bk, h) for h in heads)
        wg_np = np.stack([
            np.concatenate([q0, q1], axis=1),
            np.concatenate([k0, k1], axis=1),
            np.concatenate([q2, k2], axis=1),
        ]).astype(NPBF16)
        wv_np = np.concatenate(
            [wcols(Wv, bv, h) for h in heads], axis=1).astype(NPBF16)

        m = {
            "hidT": hidT_bf,
            "wg": np.ascontiguousarray(wg_np),
            "wv": np.ascontiguousarray(wv_np),
            "embDR": embDR_np,
            "smT": smT,
        }
        if use_m:
            mv = attention_mask[b, 0, 0].astype(np.float32)
            m["mvec"] = np.ascontiguousarray(
                np.stack([8.0 * mv, SCH_B + SCH_A * mv], axis=1))
        in_maps.append(m)

    res = run_bass_kernel_spmd(nc, in_maps, list(range(NCORES)))

    out = np.empty((B, S, HID), np.float32)
    for core in range(NCORES):
        b = core // 4
        k4 = core % 4
        r = res.results[core]["outu"].astype(np.float32)  # [128, 3, 16, 65]
        # unit u -> (lt = u//2, br = u%2); partition p -> l = lt*128 + p
        r = r.reshape(128, HPC, 8, 2, 65)
        ctx = r[..., 0:64]                        # [128, h, lt, br, 64]
        sums = r[..., 64]                         # [128, h, lt, br]
        ctx = ctx.transpose(2, 0, 1, 3, 4).reshape(S, HPC, 2, 64)
        sums = sums.transpose(2, 0, 1, 3).reshape(S, HPC, 2)
        sel = selector_outputs[b, 0, :, 0][:, None]  # [S, 1]
        cg = ctx[:, :, 0] / sums[:, :, 0:1]
        cl = ctx[:, :, 1] / sums[:, :, 1:2]
        blended = sel[:, :, None] * cl + (1.0 - sel)[:, :, None] * cg
        out[b, :, 192 * k4:192 * (k4 + 1)] = blended.reshape(S, 192)
    return out


# revision 53
# speedup vs baseline: 1.3257x; 1.2186x over previous
"""Bass/Trainium2 kernel for nn_BayesianBertSelfAttention (B=2,S=1024,HID=768,NH=12,HD=64).

Sharding: 24 (batch, head) pairs over 8 cores -> core c handles batch c//4,
heads {3k, 3k+1, 3k+2} with k = c%4.

Per-core pipeline (scoresT layout st[r, l]):
  P:  q/k projections as 3 column-packed bf16 matmul groups ([q0|q1], [k0|k1],
      [q2|k2]); k2 realigned to partitions 0:64 of a 4th slot by a gpsimd
      partition-moving DMA; q cast to fp8 and realigned to [32,2,S] DoubleRow
      layout (2 DMAs per head); v projected to va [128 r, 8 rs, 3 h, 65] bf16
      with a ones column (softmax row sums fall out of the context matmul).
  R(h): relative-position band R'[l, c] = q . E_rev via fp8 DoubleRow matmuls
      (qDR [32,2,128] x embDR [32,2,W]), one psum pair [128,640]+[128,511];
      psum -> fp8 SBUF copies split across ACT/DVE/Pool; DMA to a DRAM scratch
      with row pitch 1152 (Music-Transformer skew as a strided access pattern).
  S(h): skewed DMA read gives bias[l, r] blocks fp8. Per rs-tile: 8 fp8
      DoubleRow matmuls against [I|0] write the TRANSPOSED bias directly into
      the score psum (start), 2 bf16 qk matmuls accumulate on top (stop).
      ACT: pg = Exp(0.125*st) bf16 (global probs, unnormalized).
      DVE: d = bits(pg) - B (i16, 4x mode); e = d * smT (i16, 2x mode).
      Pool: plb = u16(e + B) -> bitcast bf16 = local probs
      (pl = pg^m via Schraudolph bits domain; validated end-to-end ~1.1% rel).
      PE: context matmuls in [l, d] orientation: lhsT = probs block [128,128],
      rhs = va slice [128, 65] -> ctx[l-part, d] psums accumulate over rs.
  Out: unnormalized cg|cl + row sums copied psum->SBUF f16, DMA per head.
      Host normalizes (sums column) and blends with selector weights.
DMAs are deliberately spread across the SP/ACT/Pool queues (transfers from
different queues overlap in the DGE/DMA-engine model).
"""

import sys

sys.path.insert(0, "/opt/trn_rl_repo")

import math
import numpy as np
import ml_dtypes
from contextlib import ExitStack

import concourse.bass as bass
import concourse.bacc as bacc
import concourse.tile as tile
from concourse import mybir
from concourse.bass_utils import run_bass_kernel_spmd
from concourse.masks import make_identity

B, S, HID, NH, HD = 2, 1024, 768, 12, 64
MAXP = 1024
NCORES = 8
HPC = 3
LTN = S // 128
BAND = 1151
PITCH = 1152
NE = 2 * MAXP - 1
NEP = 2 * MAXP      # emb plane stride padded even: odd stride + DR ifmap crashes HW

BF16 = mybir.dt.bfloat16
F16 = mybir.dt.float16
F32 = mybir.dt.float32
FP8 = mybir.dt.float8e4
U16 = mybir.dt.uint16
I16 = mybir.dt.int16
COPY = mybir.ActivationFunctionType.Copy
EXP = mybir.ActivationFunctionType.Exp
DR = mybir.MatmulPerfMode.DoubleRow

NPBF16 = ml_dtypes.bfloat16
NPFP8 = ml_dtypes.float8_e4m3

SCH_A = 128.0 / math.log(2.0)     # bf16 schraudolph slope
SCH_B = 127.0 * 128.0             # bf16 schraudolph intercept

# q/k slot layout in qkT_sb [128, 4, S]: (group, partition offset)
QG = [(0, 0), (0, 64), (2, 0)]
KG = [(1, 0), (1, 64), (3, 0)]    # k2 moved to slot 3 po 0 by a DMA

_programs = {}


def _bcast(ap, dim_count, insert_at):
    new = list(ap.ap)
    new.insert(insert_at, [0, dim_count])
    return bass.AP(tensor=ap.tensor, offset=ap.offset, ap=new)


def build_program(n_cc=6, use_m=False):
    nc = bacc.Bacc(None)
    CH = n_cc * 128

    hidT = nc.dram_tensor("hidT", [CH, S], BF16, kind="ExternalInput")
    wg = nc.dram_tensor("wg", [3, CH, 128], BF16, kind="ExternalInput")
    wv = nc.dram_tensor("wv", [CH, HPC * HD], BF16, kind="ExternalInput")
    # fp8 E bands: plane 0 = E_rev.T rows (dup at partitions 64:128 so q1's
    # partition offset matches), plane 1 = zeros (DR pairs with bcast q)
    embDR = nc.dram_tensor("embDR", [128, 2, NEP], FP8, kind="ExternalInput")
    smT = nc.dram_tensor("smT", [S, S], BF16, kind="ExternalInput")
    if use_m:
        mvec = nc.dram_tensor("mvec", [S, 2], F32, kind="ExternalInput")
    outu = nc.dram_tensor("outu", [128, HPC, 16, 65], F16, kind="ExternalOutput")
    skews = [[nc.dram_tensor(f"skew{h}_{p}", [2 * 128 * PITCH], FP8)
              for p in range(LTN // 2)] for h in range(HPC)]

    with tile.TileContext(nc) as tc, ExitStack() as ctx:
        singles = ctx.enter_context(tc.tile_pool(name="singles", bufs=1))

        hid_sb = singles.tile([128, n_cc, S], BF16)
        wg_sb = singles.tile([128, 3, n_cc, 128], BF16)
        wv_sb = singles.tile([128, n_cc, HPC * HD], BF16)
        emb_sb = singles.tile([128, 2, NEP], FP8)
        smT_sb = singles.tile([128, 8, S], BF16)
        qkT_sb = singles.tile([128, 4, S], BF16)
        qf8_sb = singles.tile([128, 2, S], FP8)   # [q0|q1] and [q2|--] fp8
        va_sb = singles.tile([128, 8, HPC, 65], BF16)

        osb = singles.tile([128, HPC, 16, 65], F16)

        wg_v = wg.rearrange("g (cc p) d -> p g cc d", p=128)
        hid_v = hidT.rearrange("(cc p) l -> p cc l", p=128)
        # SP: wg0 then even hid chunks; Pool SWDGE: odd hid chunks (all of
        # hid lands ~4us so G0 finishes early); ACT queue (engine idle
        # early): wg1, emb, wg2, wv; smT trails on Pool (needed only by S(0))
        nc.sync.dma_start(out=wg_sb[:, 0], in_=wg_v[:, 0])
        for cc in range(0, n_cc, 2):
            nc.sync.dma_start(out=hid_sb[:, cc], in_=hid_v[:, cc])
        for cc in range(1, n_cc, 2):
            nc.gpsimd.dma_start(out=hid_sb[:, cc], in_=hid_v[:, cc])
        nc.scalar.dma_start(out=wg_sb[:, 1], in_=wg_v[:, 1])
        nc.scalar.dma_start(out=emb_sb, in_=embDR[:, :, :])
        nc.scalar.dma_start(out=wg_sb[:, 2], in_=wg_v[:, 2])
        nc.scalar.dma_start(out=wv_sb,
                            in_=wv.rearrange("(cc p) d -> p cc d", p=128))
        smT_v = smT.rearrange("(rs p) l -> p rs l", p=128)
        for rsq in range(8):
            nc.sync.dma_start(out=smT_sb[:, rsq], in_=smT_v[:, rsq])
        if use_m:
            m_sb = singles.tile([128, 8, 2], F32)
            nc.gpsimd.dma_start(out=m_sb,
                                in_=mvec.rearrange("(rs p) w -> p rs w", p=128))

        identDR = singles.tile([128, 2, 128], FP8)
        make_identity(nc, identDR[:, 0, :])
        nc.vector.memset(identDR[:, 1, :], 0.0)
        nc.vector.memset(va_sb[:, :, :, 64], 1.0)

        with tc.tile_pool(name="bskp", bufs=3) as bskp, \
             tc.tile_pool(name="rtp", bufs=2) as rtp:

            rts_tiles = {}
            bsk_tiles = {}

            # fp8 q slot for the DR band matmuls: head -> (slot, po)
            Q8 = [(0, 0), (0, 64), (1, 0)]

            def emit_band_lt(pool, h, lt, on_act, tag="band"):
                e0 = 896 - lt * 128
                pr = pool.tile([128, BAND], F32, tag=tag)
                qslot, qpo8 = Q8[h]
                qblDR = _bcast(
                    qf8_sb[qpo8:qpo8 + 64, qslot, lt * 128:(lt + 1) * 128], 2, 1)
                for c0, c1 in ((0, 512), (512, 640), (640, 1024), (1024, BAND)):
                    nc.tensor.matmul(
                        pr[:, c0:c1],
                        lhsT=qblDR,
                        rhs=emb_sb[qpo8:qpo8 + 64, :, e0 + c0:e0 + c1],
                        start=True, stop=True, perf_mode=DR,
                    )
                rts = rts_tiles[h]
                # single psum->fp8 copy per lt
                if on_act:
                    nc.scalar.activation(rts[:, lt, :], pr, COPY)
                else:
                    nc.vector.tensor_copy(rts[:, lt, :], pr)
                # 2-lt-granular skew roundtrip on SP, chasing; one DRAM
                # tensor per pair so the read depends only on its own write
                if lt % 2 == 1:
                    sk = skews[h][lt // 2]
                    wview = sk[0:2 * 128 * PITCH] \
                        .rearrange("(q p c) -> p q c", c=PITCH, q=2)[:, :, 0:BAND]
                    nc.sync.dma_start(out=wview, in_=rts[:, lt - 1:lt + 1, :])
                    rview = bass.AP(tensor=sk[:].tensor, offset=127,
                                    ap=[[BAND, 128], [128 * PITCH, 2], [1, S]])
                    nc.sync.dma_start(out=bsk_tiles[h][:, lt - 1:lt + 1, :],
                                      in_=rview)

            def prep_head(h):
                rts_new = rtp.tile([128, LTN, BAND], FP8, tag="rts")
                rts_tiles[h] = rts_new
                bsk_new = bskp.tile([128, LTN, S], FP8, tag=f"bsk{h}")
                bsk_tiles[h] = bsk_new

            # -------- phase P: projections + v + head-0 band --------
            with tc.tile_pool(name="ps_pt", bufs=1, space="PSUM") as ps_pt, \
                 tc.tile_pool(name="ps_pr0", bufs=1, space="PSUM") as ps_pr0:

                def emit_G_mms(g):
                    pt = ps_pt.tile([128, S], F32, tag="pt")
                    # cc-outer so matmuls chase the hid chunk DMAs
                    for cc in range(n_cc):
                        for n in range(2):
                            nc.tensor.matmul(
                                pt[:, n * 512:(n + 1) * 512],
                                lhsT=wg_sb[:, g, cc, :],
                                rhs=hid_sb[:, cc, n * 512:(n + 1) * 512],
                                start=(cc == 0), stop=(cc == n_cc - 1),
                            )
                    return pt

                def emit_vpair(rp):
                    # v pair in a pt-ring tile: regions [0:192] and [512:704]
                    pt = ps_pt.tile([128, S], F32, tag="pt")
                    for par in range(2):
                        rs = 2 * rp + par
                        for cc in range(n_cc):
                            nc.tensor.matmul(
                                pt[:, par * 512:par * 512 + HPC * HD],
                                lhsT=hid_sb[:, cc, rs * 128:(rs + 1) * 128],
                                rhs=wv_sb[:, cc, :],
                                start=(cc == 0), stop=(cc == n_cc - 1),
                            )
                    for par in range(2):
                        pslice = pt[:, par * 512:par * 512 + HPC * HD] \
                            .rearrange("p (h d) -> p h d", d=64)
                        if rp % 2 == 0:
                            nc.scalar.activation(
                                va_sb[:, 2 * rp + par, :, 0:64], pslice, COPY)
                        else:
                            nc.vector.tensor_copy(
                                va_sb[:, 2 * rp + par, :, 0:64], pslice)

                for h in range(HPC):
                    prep_head(h)
                g0 = emit_G_mms(0)
                # qf8 cast first: it gates every band matmul
                nc.vector.tensor_copy(qf8_sb[:, 0, :], g0)
                # k0|k1 in the aux band slot so the pt ring (G0's readers)
                # never blocks it
                g1 = ps_pr0.tile([128, BAND], F32, tag="aux")
                for cc in range(n_cc):
                    for n in range(2):
                        nc.tensor.matmul(
                            g1[:, n * 512:(n + 1) * 512],
                            lhsT=wg_sb[:, 1, cc, :],
                            rhs=hid_sb[:, cc, n * 512:(n + 1) * 512],
                            start=(cc == 0), stop=(cc == n_cc - 1),
                        )
                nc.vector.tensor_copy(qkT_sb[:, 1, :], g1[:, 0:S])
                for lt in range(LTN):
                    emit_band_lt(ps_pr0, 0, lt, on_act=(lt % 2 == 0),
                                 tag=("band" if lt % 2 == 0 else "aux"))
                    if lt == 3:
                        nc.scalar.activation(qkT_sb[:, 0, :], g0, COPY)
                g2 = emit_G_mms(2)
                nc.vector.tensor_copy(qf8_sb[0:64, 1, :], g2[0:64, :])
                # band for head 1 also in P; head 2's band is produced
                # inside S(0) (psum budget: split-st S(0) frees 2 banks)
                for lt in range(LTN):
                    emit_band_lt(ps_pr0, 1, lt, on_act=(lt % 2 == 0),
                                 tag=("band" if lt % 2 == 0 else "aux"))
                    if lt == 1:
                        nc.scalar.activation(qkT_sb[:, 2, :], g2, COPY)
                for rp in range(4):
                    emit_vpair(rp)
                for lt in range(LTN):
                    emit_band_lt(ps_pr0, 2, lt, on_act=(lt % 2 == 0),
                                 tag=("band" if lt % 2 == 0 else "aux"))
                nc.gpsimd.dma_start(out=qkT_sb[0:64, 3, :],
                                    in_=qkT_sb[64:128, 2, :])

            # ----------------- phase S: scores/probs/context -----------------
            with tc.tile_pool(name="pgp", bufs=6) as pgp, \
                 tc.tile_pool(name="dp", bufs=4) as dp, \
                 tc.tile_pool(name="plp", bufs=6) as plp:

                def emit_S(ps_st, ps_cx, h, split, band_pool):
                    bsk8 = bsk_tiles[h]
                    qg, qpo = QG[h]
                    kg, kpo = KG[h]
                    qf = qkT_sb[qpo:qpo + 64, qg, :]
                    kf = qkT_sb[kpo:kpo + 64, kg, :]
                    cx0 = ps_cx.tile([128, 7, 65], F32, tag="cx0")
                    cx1 = ps_cx.tile([128, 7, 65], F32, tag="cx1")
                    cx2 = ps_cx.tile([128, 2, 65], F32, tag="cx2")
                    cx = [cx0, cx1, cx2]
                    touched = [False, False, False]

                    def emit_ctx(rs, pg, pl):
                        for u in range(16):
                            lt, br = u // 2, u % 2
                            ti, reg = u // 7, u % 7
                            lhs = (pg if br == 0 else pl)[:, lt * 128:(lt + 1) * 128]
                            nc.tensor.matmul(
                                cx[ti][:, reg, :],
                                lhsT=lhs,
                                rhs=va_sb[:, rs, h, :],
                                start=(not touched[ti]),
                                stop=(rs == 7 and (u == 6 or u == 13 or u == 15)),
                            )
                            touched[ti] = True

                    pending = {}
                    for rs in range(8):
                        pg = pgp.tile([128, S], BF16, tag="pg")
                        if split:
                            for half in range(2):
                                st = ps_st.tile([128, 512], F32, tag=f"st{half}")
                                for li in range(4):
                                    lt = half * 4 + li
                                    nc.tensor.matmul(
                                        st[:, li * 128:(li + 1) * 128],
                                        lhsT=_bcast(
                                            bsk8[:, lt, rs * 128:(rs + 1) * 128],
                                            2, 1),
                                        rhs=identDR[:, :, :],
                                        start=(li == 0), stop=False, perf_mode=DR,
                                    )
                                nc.tensor.matmul(
                                    st,
                                    lhsT=kf[:, rs * 128:(rs + 1) * 128],
                                    rhs=qf[:, half * 512:(half + 1) * 512],
                                    start=False, stop=True,
                                )
                                if use_m:
                                    nc.vector.tensor_scalar_add(
                                        st, st, m_sb[:, rs, 0:1])
                                nc.scalar.activation(
                                    pg[:, half * 512:(half + 1) * 512],
                                    st, EXP, scale=0.125)
                        else:
                            st = ps_st.tile([128, S], F32, tag="st")
                            for lt in range(LTN):
                                nc.tensor.matmul(
                                    st[:, lt * 128:(lt + 1) * 128],
                                    lhsT=_bcast(
                                        bsk8[:, lt, rs * 128:(rs + 1) * 128], 2, 1),
                                    rhs=identDR[:, :, :],
                                    start=(lt % 4 == 0), stop=False, perf_mode=DR,
                                )
                            for half in range(2):
                                nc.tensor.matmul(
                                    st[:, half * 512:(half + 1) * 512],
                                    lhsT=kf[:, rs * 128:(rs + 1) * 128],
                                    rhs=qf[:, half * 512:(half + 1) * 512],
                                    start=False, stop=True,
                                )
                            if use_m:
                                nc.vector.tensor_scalar_add(st, st, m_sb[:, rs, 0:1])
                            nc.scalar.activation(pg, st, EXP, scale=0.125)

                        # local branch: pl = pg^m in schraudolph bits domain
                        d16 = dp.tile([128, S], I16, tag="d")
                        nc.vector.tensor_scalar_add(d16, pg[:, :].bitcast(U16),
                                                    -SCH_B)
                        e16 = dp.tile([128, S], I16, tag="e")
                        nc.vector.tensor_tensor(e16, d16, smT_sb[:, rs, :],
                                                op=mybir.AluOpType.mult)
                        plb = plp.tile([128, S], U16, tag="pl")
                        if use_m:
                            nc.gpsimd.tensor_scalar_add(plb, e16, m_sb[:, rs, 1:2])
                        else:
                            nc.gpsimd.tensor_scalar_add(plb, e16, SCH_B)

                        # head 2's band interleaves with S(0)
                        if band_pool is not None:
                            emit_band_lt(band_pool, 2, rs, on_act=(rs % 2 == 0))
                        if rs >= 2:
                            emit_ctx(rs - 2, *pending[rs - 2])
                        pending[rs] = (pg, plb[:, :].bitcast(BF16))

                    emit_ctx(6, *pending[6])
                    emit_ctx(7, *pending[7])

                    nc.scalar.activation(osb[:, h, 0:7, :], cx0, COPY)
                    nc.vector.tensor_copy(osb[:, h, 7:14, :], cx1)
                    nc.vector.tensor_copy(osb[:, h, 14:16, :], cx2)
                    nc.sync.dma_start(out=outu[:, h], in_=osb[:, h])

                with tc.tile_pool(name="ps_st12", bufs=2, space="PSUM") as ps_st12, \
                     tc.tile_pool(name="ps_cx12", bufs=1, space="PSUM") as ps_cx12:
                    emit_S(ps_st12, ps_cx12, 0, split=False, band_pool=None)
                    emit_S(ps_st12, ps_cx12, 1, split=False, band_pool=None)
                    emit_S(ps_st12, ps_cx12, 2, split=False, band_pool=None)

    nc.compile()
    return nc


def _get_program(n_cc, use_m):
    key = (n_cc, use_m)
    if key not in _programs:
        _programs[key] = build_program(n_cc, use_m)
    return _programs[key]


def kernel(hidden_states, attention_mask, scaled_attention_mask, selector_outputs,
           Wq, bq, Wk, bk, Wv, bv, dist_emb):
    hidden_states = np.asarray(hidden_states, np.float32)
    attention_mask = np.asarray(attention_mask, np.float32)
    scaled_attention_mask = np.asarray(scaled_attention_mask, np.float32)
    selector_outputs = np.asarray(selector_outputs, np.float32)
    Wq, Wk, Wv = (np.asarray(x, np.float32) for x in (Wq, Wk, Wv))
    bq, bk, bv = (np.asarray(x, np.float32) for x in (bq, bk, bv))
    dist_emb = np.asarray(dist_emb, np.float32)

    use_bias = bool(np.any(bq) or np.any(bk) or np.any(bv))
    use_m = bool(np.any(attention_mask))
    n_cc = 7 if use_bias else 6
    CH = n_cc * 128
    nc = _get_program(n_cc, use_m)

    smT = np.ascontiguousarray(scaled_attention_mask[0, 0].T).astype(NPBF16)
    e_rev_t = dist_emb[::-1].T.astype(np.float32)  # [64, 2047]
    embDR_np = np.zeros((128, 2, NEP), np.float32)
    embDR_np[0:64, 0, 0:NE] = e_rev_t
    embDR_np[64:128, 0, 0:NE] = e_rev_t
    embDR_np = np.ascontiguousarray(embDR_np).astype(NPFP8)

    in_maps = []
    for core in range(NCORES):
        b = core // 4
        k4 = core % 4
        heads = [3 * k4, 3 * k4 + 1, 3 * k4 + 2]

        hidT = hidden_states[b].T
        if use_bias:
            hidT = np.concatenate(
                [hidT, np.ones((1, S), np.float32),
                 np.zeros((CH - HID - 1, S), np.float32)], axis=0)
        hidT_bf = np.ascontiguousarray(hidT).astype(NPBF16)

        def wcols(W, bvec, h):
            c = W[:, h * HD:(h + 1) * HD]
            if use_bias:
                c = np.concatenate(
                    [c, bvec[None, h * HD:(h + 1) * HD],
                     np.zeros((CH - HID - 1, HD), np.float32)], axis=0)
            return c

        q0, q1, q2 = (wcols(Wq, bq, h) for h in heads)
        k0, k1, k2 = (wcols(Wk, bk, h) for h in heads)
        wg_np = np.stack([
            np.concatenate([q0, q1], axis=1),
            np.concatenate([k0, k1], axis=1),
            np.concatenate([q2, k2], axis=1),
        ]).astype(NPBF16)
        wv_np = np.concatenate(
            [wcols(Wv, bv, h) for h in heads], axis=1).astype(NPBF16)

        m = {
            "hidT": hidT_bf,
            "wg": np.ascontiguousarray(wg_np),
            "wv": np.ascontiguousarray(wv_np),
            "embDR": embDR_np,
            "smT": smT,
        }
        if use_m:
            mv = attention_mask[b, 0, 0].astype(np.float32)
            m["mvec"] = np.ascontiguousarray(
                np.stack([8.0 * mv, SCH_B + SCH_A * mv], axis=1))
        in_maps.append(m)

    res = run_bass_kernel_spmd(nc, in_maps, list(range(NCORES)))

    out = np.empty((B, S, HID), np.float32)
    for core in range(NCORES):
        b = core // 4
        k4 = core % 4
        r = res.results[core]["outu"].astype(np.float32)  # [128, 3, 16, 65]
        # unit u -> (lt = u//2, br = u%2); partition p -> l = lt*128 + p
        r = r.reshape(128, HPC, 8, 2, 65)
        ctx = r[..., 0:64]                        # [128, h, lt, br, 64]
        sums = r[..., 64]                         # [128, h, lt, br]
        ctx = ctx.transpose(2, 0, 1, 3, 4).reshape(S, HPC, 2, 64)
        sums = sums.transpose(2, 0, 1, 3).reshape(S, HPC, 2)
        sel = selector_outputs[b, 0, :, 0][:, None]  # [S, 1]
        cg = ctx[:, :, 0] / sums[:, :, 0:1]
        cl = ctx[:, :, 1] / sums[:, :, 1:2]
        blended = sel[:, :, None] * cl + (1.0 - sel)[:, :, None] * cg
        out[b, :, 192 * k4:192 * (k4 + 1)] = blended.reshape(S, 192)
    return out


# revision 59
# speedup vs baseline: 1.3290x; 1.0025x over previous
"""Bass/Trainium2 kernel for nn_BayesianBertSelfAttention (B=2,S=1024,HID=768,NH=12,HD=64).

Sharding: 24 (batch, head) pairs over 8 cores -> core c handles batch c//4,
heads {3k, 3k+1, 3k+2} with k = c%4.

Per-core pipeline (scoresT layout st[r, l]):
  P:  q/k projections as 3 column-packed bf16 matmul groups ([q0|q1], [k0|k1],
      [q2|k2]); k2 realigned to partitions 0:64 of a 4th slot by a gpsimd
      partition-moving DMA; q cast to fp8 and realigned to [32,2,S] DoubleRow
      layout (2 DMAs per head); v projected to va [128 r, 8 rs, 3 h, 65] bf16
      with a ones column (softmax row sums fall out of the context matmul).
  R(h): relative-position band R'[l, c] = q . E_rev via fp8 DoubleRow matmuls
      (qDR [32,2,128] x embDR [32,2,W]), one psum pair [128,640]+[128,511];
      psum -> fp8 SBUF copies split across ACT/DVE/Pool; DMA to a DRAM scratch
      with row pitch 1152 (Music-Transformer skew as a strided access pattern).
  S(h): skewed DMA read gives bias[l, r] blocks fp8. Per rs-tile: 8 fp8
      DoubleRow matmuls against [I|0] write the TRANSPOSED bias directly into
      the score psum (start), 2 bf16 qk matmuls accumulate on top (stop).
      ACT: pg = Exp(0.125*st) bf16 (global probs, unnormalized).
      DVE: d = bits(pg) - B (i16, 4x mode); e = d * smT (i16, 2x mode).
      Pool: plb = u16(e + B) -> bitcast bf16 = local probs
      (pl = pg^m via Schraudolph bits domain; validated end-to-end ~1.1% rel).
      PE: context matmuls in [l, d] orientation: lhsT = probs block [128,128],
      rhs = va slice [128, 65] -> ctx[l-part, d] psums accumulate over rs.
  Out: unnormalized cg|cl + row sums copied psum->SBUF f16, DMA per head.
      Host normalizes (sums column) and blends with selector weights.
DMAs are deliberately spread across the SP/ACT/Pool queues (transfers from
different queues overlap in the DGE/DMA-engine model).
"""

import sys

sys.path.insert(0, "/opt/trn_rl_repo")

import math
import numpy as np
import ml_dtypes
from contextlib import ExitStack

import concourse.bass as bass
import concourse.bacc as bacc
import concourse.tile as tile
from concourse import mybir
from concourse.bass_utils import run_bass_kernel_spmd
from concourse.masks import make_identity

B, S, HID, NH, HD = 2, 1024, 768, 12, 64
MAXP = 1024
NCORES = 8
HPC = 3
LTN = S // 128
BAND = 1151
PITCH = 1152
NE = 2 * MAXP - 1
NEP = 2 * MAXP      # emb plane stride padded even: odd stride + DR ifmap crashes HW

BF16 = mybir.dt.bfloat16
F16 = mybir.dt.float16
F32 = mybir.dt.float32
FP8 = mybir.dt.float8e4
U16 = mybir.dt.uint16
I16 = mybir.dt.int16
COPY = mybir.ActivationFunctionType.Copy
EXP = mybir.ActivationFunctionType.Exp
DR = mybir.MatmulPerfMode.DoubleRow

NPBF16 = ml_dtypes.bfloat16
NPFP8 = ml_dtypes.float8_e4m3

SCH_A = 128.0 / math.log(2.0)     # bf16 schraudolph slope
SCH_B = 127.0 * 128.0             # bf16 schraudolph intercept

# q/k slot layout in qkT_sb [128, 4, S]: (group, partition offset)
QG = [(0, 0), (0, 64), (2, 0)]
KG = [(1, 0), (1, 64), (3, 0)]    # k2 moved to slot 3 po 0 by a DMA

_programs = {}


def _bcast(ap, dim_count, insert_at):
    new = list(ap.ap)
    new.insert(insert_at, [0, dim_count])
    return bass.AP(tensor=ap.tensor, offset=ap.offset, ap=new)


def build_program(n_cc=6, use_m=False):
    nc = bacc.Bacc(None)
    CH = n_cc * 128

    hidT = nc.dram_tensor("hidT", [CH, S], BF16, kind="ExternalInput")
    wg = nc.dram_tensor("wg", [3, CH, 128], BF16, kind="ExternalInput")
    wv = nc.dram_tensor("wv", [CH, HPC * HD], BF16, kind="ExternalInput")
    # fp8 E bands: plane 0 = E_rev.T rows (dup at partitions 64:128 so q1's
    # partition offset matches), plane 1 = zeros (DR pairs with bcast q)
    embDR = nc.dram_tensor("embDR", [128, 2, NEP], FP8, kind="ExternalInput")
    smT = nc.dram_tensor("smT", [S, S], BF16, kind="ExternalInput")
    if use_m:
        mvec = nc.dram_tensor("mvec", [S, 2], F32, kind="ExternalInput")
    outu = nc.dram_tensor("outu", [128, HPC, 16, 65], F16, kind="ExternalOutput")
    skews = [[nc.dram_tensor(f"skew{h}_{p}", [2 * 128 * PITCH], FP8)
              for p in range(LTN // 2)] for h in range(HPC)]

    with tile.TileContext(nc) as tc, ExitStack() as ctx:
        singles = ctx.enter_context(tc.tile_pool(name="singles", bufs=1))

        hid_sb = singles.tile([128, n_cc, S], BF16)
        wg_sb = singles.tile([128, 3, n_cc, 128], BF16)
        wv_sb = singles.tile([128, n_cc, HPC * HD], BF16)
        emb_sb = singles.tile([128, 2, NEP], FP8)
        smT_sb = singles.tile([128, 8, S], BF16)
        qkT_sb = singles.tile([128, 4, S], BF16)
        qf8_sb = singles.tile([128, 2, S], FP8)   # [q0|q1] and [q2|--] fp8
        va_sb = singles.tile([128, 8, HPC, 65], BF16)

        osb = singles.tile([128, HPC, 16, 65], F16)

        wg_v = wg.rearrange("g (cc p) d -> p g cc d", p=128)
        hid_v = hidT.rearrange("(cc p) l -> p cc l", p=128)
        # SP: wg0 then even hid chunks; Pool SWDGE: odd hid chunks (all of
        # hid lands ~4us so G0 finishes early); ACT queue (engine idle
        # early): wg1, emb, wg2, wv; smT trails on Pool (needed only by S(0))
        nc.sync.dma_start(out=wg_sb[:, 0, 0:3], in_=wg_v[:, 0, 0:3])
        nc.scalar.dma_start(out=wg_sb[:, 0, 3:n_cc], in_=wg_v[:, 0, 3:n_cc])
        for cc in range(0, n_cc, 2):
            nc.sync.dma_start(out=hid_sb[:, cc], in_=hid_v[:, cc])
        for cc in range(1, n_cc, 2):
            nc.gpsimd.dma_start(out=hid_sb[:, cc], in_=hid_v[:, cc])
        nc.scalar.dma_start(out=wg_sb[:, 1], in_=wg_v[:, 1])
        nc.scalar.dma_start(out=emb_sb, in_=embDR[:, :, :])
        nc.scalar.dma_start(out=wg_sb[:, 2], in_=wg_v[:, 2])
        nc.scalar.dma_start(out=wv_sb,
                            in_=wv.rearrange("(cc p) d -> p cc d", p=128))
        smT_v = smT.rearrange("(rs p) l -> p rs l", p=128)
        for rsq in range(8):
            nc.sync.dma_start(out=smT_sb[:, rsq], in_=smT_v[:, rsq])
        if use_m:
            m_sb = singles.tile([128, 8, 2], F32)
            nc.gpsimd.dma_start(out=m_sb,
                                in_=mvec.rearrange("(rs p) w -> p rs w", p=128))

        identDR = singles.tile([128, 2, 128], FP8)
        make_identity(nc, identDR[:, 0, :])
        nc.vector.memset(identDR[:, 1, :], 0.0)
        nc.vector.memset(va_sb[:, :, :, 64], 1.0)

        with tc.tile_pool(name="bskp", bufs=1) as bskp, \
             tc.tile_pool(name="rtp", bufs=2) as rtp:

            rts_tiles = {}
            bsk_tiles = {}

            # fp8 q slot for the DR band matmuls: head -> (slot, po)
            Q8 = [(0, 0), (0, 64), (1, 0)]

            def emit_band_lt(pool, h, lt, on_act, tag="band"):
                e0 = 896 - lt * 128
                pr = pool.tile([128, BAND], F32, tag=tag)
                qslot, qpo8 = Q8[h]
                qblDR = _bcast(
                    qf8_sb[qpo8:qpo8 + 64, qslot, lt * 128:(lt + 1) * 128], 2, 1)
                for c0, c1 in ((0, 512), (512, 640), (640, 1024), (1024, BAND)):
                    nc.tensor.matmul(
                        pr[:, c0:c1],
                        lhsT=qblDR,
                        rhs=emb_sb[qpo8:qpo8 + 64, :, e0 + c0:e0 + c1],
                        start=True, stop=True, perf_mode=DR,
                    )
                rts = rts_tiles[h]
                # single psum->fp8 copy per lt
                if on_act:
                    nc.scalar.activation(rts[:, lt, :], pr, COPY)
                else:
                    nc.vector.tensor_copy(rts[:, lt, :], pr)
                # 2-lt-granular skew roundtrip on SP, chasing; one DRAM
                # tensor per pair so the read depends only on its own write
                if lt % 2 == 1:
                    sk = skews[h][lt // 2]
                    wview = sk[0:2 * 128 * PITCH] \
                        .rearrange("(q p c) -> p q c", c=PITCH, q=2)[:, :, 0:BAND]
                    nc.sync.dma_start(out=wview, in_=rts[:, lt - 1:lt + 1, :])
                    rview = bass.AP(tensor=sk[:].tensor, offset=127,
                                    ap=[[BAND, 128], [128 * PITCH, 2], [1, S]])
                    nc.sync.dma_start(out=bsk_tiles[h][:, lt - 1:lt + 1, :],
                                      in_=rview)

            def prep_head(h):
                rts_new = rtp.tile([128, LTN, BAND], FP8, tag="rts")
                rts_tiles[h] = rts_new
                bsk_new = bskp.tile([128, LTN, S], FP8, tag=f"bsk{h}")
                bsk_tiles[h] = bsk_new

            # -------- phase P: projections + v + head-0 band --------
            with tc.tile_pool(name="ps_pt", bufs=1, space="PSUM") as ps_pt, \
                 tc.tile_pool(name="ps_pr0", bufs=1, space="PSUM") as ps_pr0:

                def emit_G_mms(g):
                    pt = ps_pt.tile([128, S], F32, tag="pt")
                    # cc-outer so matmuls chase the hid chunk DMAs
                    for cc in range(n_cc):
                        for n in range(2):
                            nc.tensor.matmul(
                                pt[:, n * 512:(n + 1) * 512],
                                lhsT=wg_sb[:, g, cc, :],
                                rhs=hid_sb[:, cc, n * 512:(n + 1) * 512],
                                start=(cc == 0), stop=(cc == n_cc - 1),
                            )
                    return pt

                def emit_vpair(rp):
                    # v pair in a pt-ring tile: regions [0:192] and [512:704]
                    pt = ps_pt.tile([128, S], F32, tag="pt")
                    for par in range(2):
                        rs = 2 * rp + par
                        for cc in range(n_cc):
                            nc.tensor.matmul(
                                pt[:, par * 512:par * 512 + HPC * HD],
                                lhsT=hid_sb[:, cc, rs * 128:(rs + 1) * 128],
                                rhs=wv_sb[:, cc, :],
                                start=(cc == 0), stop=(cc == n_cc - 1),
                            )
                    for par in range(2):
                        pslice = pt[:, par * 512:par * 512 + HPC * HD] \
                            .rearrange("p (h d) -> p h d", d=64)
                        if rp % 2 == 0:
                            nc.scalar.activation(
                                va_sb[:, 2 * rp + par, :, 0:64], pslice, COPY)
                        else:
                            nc.vector.tensor_copy(
                                va_sb[:, 2 * rp + par, :, 0:64], pslice)

                for h in range(HPC):
                    prep_head(h)
                g0 = emit_G_mms(0)
                # qf8 cast first: it gates every band matmul
                nc.vector.tensor_copy(qf8_sb[:, 0, :], g0)
                # k0|k1 in the aux band slot so the pt ring (G0's readers)
                # never blocks it
                g1 = ps_pr0.tile([128, BAND], F32, tag="aux")
                for cc in range(n_cc):
                    for n in range(2):
                        nc.tensor.matmul(
                            g1[:, n * 512:(n + 1) * 512],
                            lhsT=wg_sb[:, 1, cc, :],
                            rhs=hid_sb[:, cc, n * 512:(n + 1) * 512],
                            start=(cc == 0), stop=(cc == n_cc - 1),
                        )
                nc.vector.tensor_copy(qkT_sb[:, 1, :], g1[:, 0:S])
                for lt in range(LTN):
                    emit_band_lt(ps_pr0, 0, lt, on_act=(lt % 2 == 0),
                                 tag=("band" if lt % 2 == 0 else "aux"))
                    if lt == 3:
                        nc.scalar.activation(qkT_sb[:, 0, :], g0, COPY)
                g2 = emit_G_mms(2)
                nc.vector.tensor_copy(qf8_sb[0:64, 1, :], g2[0:64, :])
                # band for head 1 also in P; head 2's band is produced
                # inside S(0) (psum budget: split-st S(0) frees 2 banks)
                for lt in range(LTN):
                    emit_band_lt(ps_pr0, 1, lt, on_act=(lt % 2 == 0),
                                 tag=("band" if lt % 2 == 0 else "aux"))
                    if lt == 1:
                        nc.scalar.activation(qkT_sb[:, 2, :], g2, COPY)
                for rp in range(4):
                    emit_vpair(rp)
                for lt in range(LTN):
                    emit_band_lt(ps_pr0, 2, lt, on_act=(lt % 2 == 0),
                                 tag=("band" if lt % 2 == 0 else "aux"))
                nc.gpsimd.dma_start(out=qkT_sb[0:64, 3, :],
                                    in_=qkT_sb[64:128, 2, :])

            # ----------------- phase S: scores/probs/context -----------------
            with tc.tile_pool(name="pgp", bufs=6) as pgp, \
                 tc.tile_pool(name="dp", bufs=4) as dp, \
                 tc.tile_pool(name="plp", bufs=6) as plp:

                def emit_S(ps_st, ps_cx, h, split, band_pool):
                    bsk8 = bsk_tiles[h]
                    qg, qpo = QG[h]
                    kg, kpo = KG[h]
                    qf = qkT_sb[qpo:qpo + 64, qg, :]
                    kf = qkT_sb[kpo:kpo + 64, kg, :]
                    cx0 = ps_cx.tile([128, 7, 65], F32, tag="cx0")
                    cx1 = ps_cx.tile([128, 7, 65], F32, tag="cx1")
                    cx2 = ps_cx.tile([128, 2, 65], F32, tag="cx2")
                    cx = [cx0, cx1, cx2]
                    touched = [False, False, False]

                    def emit_ctx(rs, pg, pl):
                        for u in range(16):
                            lt, br = u // 2, u % 2
                            ti, reg = u // 7, u % 7
                            lhs = (pg if br == 0 else pl)[:, lt * 128:(lt + 1) * 128]
                            nc.tensor.matmul(
                                cx[ti][:, reg, :],
                                lhsT=lhs,
                                rhs=va_sb[:, rs, h, :],
                                start=(not touched[ti]),
                                stop=(rs == 7 and (u == 6 or u == 13 or u == 15)),
                            )
                            touched[ti] = True

                    pending = {}
                    for rs in range(8):
                        pg = pgp.tile([128, S], BF16, tag="pg")
                        if split:
                            for half in range(2):
                                st = ps_st.tile([128, 512], F32, tag=f"st{half}")
                                for li in range(4):
                                    lt = half * 4 + li
                                    nc.tensor.matmul(
                                        st[:, li * 128:(li + 1) * 128],
                                        lhsT=_bcast(
                                            bsk8[:, lt, rs * 128:(rs + 1) * 128],
                                            2, 1),
                                        rhs=identDR[:, :, :],
                                        start=(li == 0), stop=False, perf_mode=DR,
                                    )
                                nc.tensor.matmul(
                                    st,
                                    lhsT=kf[:, rs * 128:(rs + 1) * 128],
                                    rhs=qf[:, half * 512:(half + 1) * 512],
                                    start=False, stop=True,
                                )
                                if use_m:
                                    nc.vector.tensor_scalar_add(
                                        st, st, m_sb[:, rs, 0:1])
                                nc.scalar.activation(
                                    pg[:, half * 512:(half + 1) * 512],
                                    st, EXP, scale=0.125)
                        else:
                            st = ps_st.tile([128, S], F32, tag="st")
                            for lt in range(LTN):
                                nc.tensor.matmul(
                                    st[:, lt * 128:(lt + 1) * 128],
                                    lhsT=_bcast(
                                        bsk8[:, lt, rs * 128:(rs + 1) * 128], 2, 1),
                                    rhs=identDR[:, :, :],
                                    start=(lt % 4 == 0), stop=False, perf_mode=DR,
                                )
                            for half in range(2):
                                nc.tensor.matmul(
                                    st[:, half * 512:(half + 1) * 512],
                                    lhsT=kf[:, rs * 128:(rs + 1) * 128],
                                    rhs=qf[:, half * 512:(half + 1) * 512],
                                    start=False, stop=True,
                                )
                            if use_m:
                                nc.vector.tensor_scalar_add(st, st, m_sb[:, rs, 0:1])
                            nc.scalar.activation(pg, st, EXP, scale=0.125)

                        # local branch: pl = pg^m in schraudolph bits domain
                        d16 = dp.tile([128, S], I16, tag="d")
                        nc.vector.tensor_scalar_add(d16, pg[:, :].bitcast(U16),
                                                    -SCH_B)
                        e16 = dp.tile([128, S], I16, tag="e")
                        nc.vector.tensor_tensor(e16, d16, smT_sb[:, rs, :],
                                                op=mybir.AluOpType.mult)
                        plb = plp.tile([128, S], U16, tag="pl")
                        if use_m:
                            nc.gpsimd.tensor_scalar_add(plb, e16, m_sb[:, rs, 1:2])
                        else:
                            nc.gpsimd.tensor_scalar_add(plb, e16, SCH_B)

                        # head 2's band interleaves with S(0)
                        if band_pool is not None:
                            emit_band_lt(band_pool, 2, rs, on_act=(rs % 2 == 0))
                        if rs >= 1:
                            emit_ctx(rs - 1, *pending[rs - 1])
                        pending[rs] = (pg, plb[:, :].bitcast(BF16))

                    emit_ctx(7, *pending[7])

                    nc.scalar.activation(osb[:, h, 0:7, :], cx0, COPY)
                    nc.vector.tensor_copy(osb[:, h, 7:14, :], cx1)
                    nc.vector.tensor_copy(osb[:, h, 14:16, :], cx2)
                    nc.sync.dma_start(out=outu[:, h], in_=osb[:, h])

                with tc.tile_pool(name="ps_st12", bufs=2, space="PSUM") as ps_st12, \
                     tc.tile_pool(name="ps_cx12", bufs=1, space="PSUM") as ps_cx12:
                    emit_S(ps_st12, ps_cx12, 0, split=False, band_pool=None)
                    emit_S(ps_st12, ps_cx12, 1, split=False, band_pool=None)
                    emit_S(ps_st12, ps_cx12, 2, split=False, band_pool=None)

    nc.compile()
    return nc


def _get_program(n_cc, use_m):
    key = (n_cc, use_m)
    if key not in _programs:
        _programs[key] = build_program(n_cc, use_m)
    return _programs[key]


def kernel(hidden_states, attention_mask, scaled_attention_mask, selector_outputs,
           Wq, bq, Wk, bk, Wv, bv, dist_emb):
    hidden_states = np.asarray(hidden_states, np.float32)
    attention_mask = np.asarray(attention_mask, np.float32)
    scaled_attention_mask = np.asarray(scaled_attention_mask, np.float32)
    selector_outputs = np.asarray(selector_outputs, np.float32)
    Wq, Wk, Wv = (np.asarray(x, np.float32) for x in (Wq, Wk, Wv))
    bq, bk, bv = (np.asarray(x, np.float32) for x in (bq, bk, bv))
    dist_emb = np.asarray(dist_emb, np.float32)

    use_bias = bool(np.any(bq) or np.any(bk) or np.any(bv))
    use_m = bool(np.any(attention_mask))
    n_cc = 7 if use_bias else 6
    CH = n_cc * 128
    nc = _get_program(n_cc, use_m)

    smT = np.ascontiguousarray(scaled_attention_mask[0, 0].T).astype(NPBF16)
    e_rev_t = dist_emb[::-1].T.astype(np.float32)  # [64, 2047]
    embDR_np = np.zeros((128, 2, NEP), np.float32)
    embDR_np[0:64, 0, 0:NE] = e_rev_t
    embDR_np[64:128, 0, 0:NE] = e_rev_t
    embDR_np = np.ascontiguousarray(embDR_np).astype(NPFP8)

    in_maps = []
    for core in range(NCORES):
        b = core // 4
        k4 = core % 4
        heads = [3 * k4, 3 * k4 + 1, 3 * k4 + 2]

        hidT = hidden_states[b].T
        if use_bias:
            hidT = np.concatenate(
                [hidT, np.ones((1, S), np.float32),
                 np.zeros((CH - HID - 1, S), np.float32)], axis=0)
        hidT_bf = np.ascontiguousarray(hidT).astype(NPBF16)

        def wcols(W, bvec, h):
            c = W[:, h * HD:(h + 1) * HD]
            if use_bias:
                c = np.concatenate(
                    [c, bvec[None, h * HD:(h + 1) * HD],
                     np.zeros((CH - HID - 1, HD), np.float32)], axis=0)
            return c

        q0, q1, q2 = (wcols(Wq, bq, h) for h in heads)
        k0, k1, k2 = (wcols(Wk, bk, h) for h in heads)
        wg_np = np.stack([
            np.concatenate([q0, q1], axis=1),
            np.concatenate([k0, k1], axis=1),
            np.concatenate([q2, k2], axis=1),
        ]).astype(NPBF16)
        wv_np = np.concatenate(
            [wcols(Wv, bv, h) for h in heads], axis=1).astype(NPBF16)

        m = {
            "hidT": hidT_bf,
            "wg": np.ascontiguousarray(wg_np),
            "wv": np.ascontiguousarray(wv_np),
            "embDR": embDR_np,
            "smT": smT,
        }
        if use_m:
            mv = attention_mask[b, 0, 0].astype(np.float32)
            m["mvec"] = np.ascontiguousarray(
                np.stack([8.0 * mv, SCH_B + SCH_A * mv], axis=1))
        in_maps.append(m)

    res = run_bass_kernel_spmd(nc, in_maps, list(range(NCORES)))

    out = np.empty((B, S, HID), np.float32)
    for core in range(NCORES):
        b = core // 4
        k4 = core % 4
        r = res.results[core]["outu"].astype(np.float32)  # [128, 3, 16, 65]
        # unit u -> (lt = u//2, br = u%2); partition p -> l = lt*128 + p
        r = r.reshape(128, HPC, 8, 2, 65)
        ctx = r[..., 0:64]                        # [128, h, lt, br, 64]
        sums = r[..., 64]                         # [128, h, lt, br]
        ctx = ctx.transpose(2, 0, 1, 3, 4).reshape(S, HPC, 2, 64)
        sums = sums.transpose(2, 0, 1, 3).reshape(S, HPC, 2)
        sel = selector_outputs[b, 0, :, 0][:, None]  # [S, 1]
        cg = ctx[:, :, 0] / sums[:, :, 0:1]
        cl = ctx[:, :, 1] / sums[:, :, 1:2]
        blended = sel[:, :, None] * cl + (1.0 - sel)[:, :, None] * cg
        out[b, :, 192 * k4:192 * (k4 + 1)] = blended.reshape(S, 192)
    return out
